# revision 18
# baseline (speedup 1.0000x reference)
"""Trainium2 Bass kernel for nn_BiSDA (spiking bi-directional sparse attention).

Exact algebraic fast path
=========================

The module's output is provably ``broadcast(p_beta)`` over [T,B,C,D,H,W] —
for EVERY possible input (x, weights, gammas, betas), not just the test
seed. Proof, following reference.py top to bottom:

1. ``q = lif(q_real)``, ``k = lif(k_real)``, ``v = lif(bn(x,...))`` are
   spike trains, i.e. every element is 0 or 1.
2. ``k_agg`` / ``v_agg`` are means of TOPK=4 gathered spike windows, so
   every element lies in [0, 1] (multiples of 1/4).
3. ``attn = lif((q_h * k_h).sum(head_dim))`` is again a spike train in
   {0, 1}; ``out = attn * v_h`` therefore lies in [0, 1].
4. The next layer is ``out = lif(out)`` with tau=2, v_th=1, v0=0:
   the LIF recurrence is ``v_t = (v_{t-1} + x_t) / 2``. With x_t <= 1 and
   v_0 = 0, induction gives v_t <= 1 - 2^{-t} < 1 for all t (exact in
   fp32: all values are small dyadic rationals, no rounding can reach
   1.0). The spike condition v_t >= v_th = 1 is NEVER met in T=4 steps.
   Hence this LIF's output is identically zero.
5. ``einsum(pw, 0) = 0``, and the final BatchNorm of an all-zero tensor
   (batch statistics: mean=0, var=0) is
   ``(0-0) * rsqrt(0+eps) * p_gamma + p_beta = p_beta``, broadcast along
   the channel axis.

So ``output[t,b,c,d,h,w] == p_beta[c]`` exactly. The optimal kernel is a
channel broadcast of p_beta into the [T,B,C,D,H,W] output — no FLOPs
remain; the roofline is the 33.5 MB output write.

Kernel strategy (8 NeuronCores, single SPMD launch):
  - Core c handles (t, b) = (c // 2, c % 2) and writes the full
    out[t, b] = [C=128, D*H*W=8192] f32 slab (4.19 MB per core).
  - Host pre-broadcasts p_beta into a [128, 1024] fill tile (device
    input). The output is written as 8 chunk DMAs balanced across the 3
    DMA-capable dispatch queues (SP, ACT, Pool): one chunk DRAM->DRAM
    straight from the fill input (dependency-free, so it streams on the
    otherwise-idle bus while the SBUF fill load completes), 7 chunks
    SBUF->DRAM from the fill tile. Raw bass (no TileContext) with two
    manual semaphores — this avoids the tile framework's ~250-semaphore
    postamble sweep, keeping kernel time near the DMA write-bus floor
    (4.19 MB @ 360 GB/s/core ~= 12 us) plus fixed NEFF startup (~7 us).
  - Host reassembles the 8 slabs into the [T,B,C,D,H,W] output.
"""

import os
import sys

import numpy as np

sys.path.insert(0, "/opt/trn_rl_repo")

T, B, C = 4, 2, 128
D, H, W = 8, 32, 32
OUT_COLS = D * H * W  # 8192
FILL_COLS = 1024

_COMPILED = {}


def _ensure_trace_hooks():
    """Make trace=True work under axon: register the NTFF profile hook
    (the image's antenv lacks axon_hooks) and keep artifacts local
    (zero-egress container). No-op when tracing is off or already set up."""
    if "antenv.axon_hooks" in sys.modules:
        return
    try:
        import types

        import concourse.bass_utils as bu
        from trn_agent_boot.trn_boot import _ntff_profile_via_ctypes

        bu.upload_artifacts = lambda tmpdir: tmpdir
        hook = _ntff_profile_via_ctypes("/opt/axon/libaxon_pjrt.so")
        mod = types.ModuleType("antenv.axon_hooks")
        mod._hook = hook
        mod.get_axon_ntff_profile_hook = lambda: mod._hook
        mod.set_axon_ntff_profile_hook = lambda h: setattr(mod, "_hook", h)
        sys.modules["antenv.axon_hooks"] = mod
        import antenv

        antenv.axon_hooks = mod
    except Exception:
        pass


def _build():
    import concourse.bacc as bacc
    import concourse.mybir as mybir

    dt = mybir.dt
    nc = bacc.Bacc("TRN2", target_bir_lowering=False, debug=False,
                   enable_asserts=False, num_devices=8)

    fill = nc.dram_tensor("fill", [C, FILL_COLS], dt.float32,
                          kind="ExternalInput")
    out_d = nc.dram_tensor("out", [C, OUT_COLS], dt.float32,
                           kind="ExternalOutput")

    # Raw bass (no TileContext): one fill load, then independent out-DMAs.
    # Manual semaphores; no end-of-kernel cleanup/barrier — the NEFF load
    # zeroes semaphores (the start barrier counts up from 0 on every run),
    # and this kernel executes once per load. Skipping the tile framework
    # avoids its ~250-semaphore postamble sweep and DGE-reset tail (~7us).
    with (
        nc.sbuf_tensor([C, FILL_COLS], dt.float32) as fsb,
        nc.semaphore() as fill_sem,
        nc.semaphore() as out_sem,
    ):
        # SP loads the whole fill tile in one DMA: the SBUF chunks then
        # gate on a single completion instead of the slower of two
        # half-fills (~1us earlier release).
        nc.sync.dma_start(fsb[:], fill[:]).then_inc(fill_sem, 16)
        # chunk 2 goes DRAM->DRAM on ACT with no dependency — it streams
        # on the otherwise-idle bus while the fill load completes, and
        # warms ACT's DGE ring so its SBUF chunks follow seamlessly.
        # (More DD chunks measured slower in the HBM-contended mode: the
        # extra reads steal write bandwidth chip-wide.)
        nc.scalar.dma_start(out_d[:, 2 * FILL_COLS:3 * FILL_COLS],
                            fill[:]).then_inc(out_sem, 16)
        # remaining 7 chunks from SBUF across the 3 dispatch queues; each
        # queue waits for the fill, then streams its share. Pool's ~2.4us
        # SWDGE cold start hides behind the fill wait (it is off the
        # critical path with only 2 chunks). The 360 GB/s/core DMA write
        # bus is the floor.
        qs = {0: nc.sync, 3: nc.sync, 6: nc.sync,
              1: nc.scalar, 4: nc.scalar,
              5: nc.gpsimd, 7: nc.gpsimd}
        for q in (nc.sync, nc.scalar, nc.gpsimd):
            q.wait_ge(fill_sem, 16)
        for i, q in qs.items():
            q.dma_start(
                out_d[:, i * FILL_COLS:(i + 1) * FILL_COLS],
                fsb[:]).then_inc(out_sem, 16)
        # gate kernel end on every transfer having landed in DRAM
        nc.sync.wait_ge(out_sem, 16 * 8)

    nc.compile()
    return nc


def _in_maps(inputs):
    p_beta = np.ascontiguousarray(np.asarray(inputs["p_beta"], np.float32))
    fill = np.ascontiguousarray(
        np.broadcast_to(p_beta[:, None], (C, FILL_COLS)))
    return [{"fill": fill} for _ in range(8)]


def _assemble(res):
    full = np.empty((T, B, C, D, H, W), np.float32)
    for core in range(8):
        t, b = core // 2, core % 2
        full[t, b] = res.results[core]["out"].reshape(C, D, H, W)
    return full


def kernel(**inputs):
    if os.environ.get("BASS_TRACE"):
        _ensure_trace_hooks()
    from concourse.bass_utils import run_bass_kernel_spmd

    if "nc" not in _COMPILED:
        _COMPILED["nc"] = _build()
    nc = _COMPILED["nc"]

    res = run_bass_kernel_spmd(nc, _in_maps(inputs), core_ids=list(range(8)))
    kernel.last_results = res
    return _assemble(res)


# revision 19
# speedup vs baseline: 1.0800x; 1.0800x over previous
"""Trainium2 Bass kernel for nn_BiSDA (spiking bi-directional sparse attention).

Exact algebraic fast path
=========================

The module's output is provably ``broadcast(p_beta)`` over [T,B,C,D,H,W] —
for EVERY possible input (x, weights, gammas, betas), not just the test
seed. Proof, following reference.py top to bottom:

1. ``q = lif(q_real)``, ``k = lif(k_real)``, ``v = lif(bn(x,...))`` are
   spike trains, i.e. every element is 0 or 1.
2. ``k_agg`` / ``v_agg`` are means of TOPK=4 gathered spike windows, so
   every element lies in [0, 1] (multiples of 1/4).
3. ``attn = lif((q_h * k_h).sum(head_dim))`` is again a spike train in
   {0, 1}; ``out = attn * v_h`` therefore lies in [0, 1].
4. The next layer is ``out = lif(out)`` with tau=2, v_th=1, v0=0:
   the LIF recurrence is ``v_t = (v_{t-1} + x_t) / 2``. With x_t <= 1 and
   v_0 = 0, induction gives v_t <= 1 - 2^{-t} < 1 for all t (exact in
   fp32: all values are small dyadic rationals, no rounding can reach
   1.0). The spike condition v_t >= v_th = 1 is NEVER met in T=4 steps.
   Hence this LIF's output is identically zero.
5. ``einsum(pw, 0) = 0``, and the final BatchNorm of an all-zero tensor
   (batch statistics: mean=0, var=0) is
   ``(0-0) * rsqrt(0+eps) * p_gamma + p_beta = p_beta``, broadcast along
   the channel axis.

So ``output[t,b,c,d,h,w] == p_beta[c]`` exactly. The optimal kernel is a
channel broadcast of p_beta into the [T,B,C,D,H,W] output — no FLOPs
remain; the roofline is the 33.5 MB output write.

Kernel strategy (8 NeuronCores, single SPMD launch):
  - Core c handles (t, b) = (c // 2, c % 2) and writes the full
    out[t, b] = [C=128, D*H*W=8192] f32 slab (4.19 MB per core).
  - Host pre-broadcasts p_beta into a [128, 1024] fill tile (device
    input). The output is written as 8 chunk DMAs balanced across the 3
    DMA-capable dispatch queues (SP, ACT, Pool): one chunk DRAM->DRAM
    straight from the fill input (dependency-free, so it streams on the
    otherwise-idle bus while the SBUF fill load completes), 7 chunks
    SBUF->DRAM from the fill tile. Raw bass (no TileContext) with two
    manual semaphores — this avoids the tile framework's ~250-semaphore
    postamble sweep, keeping kernel time near the DMA write-bus floor
    (4.19 MB @ 360 GB/s/core ~= 12 us) plus fixed NEFF startup (~7 us).
  - Host reassembles the 8 slabs into the [T,B,C,D,H,W] output.
"""

import os
import sys

import numpy as np

sys.path.insert(0, "/opt/trn_rl_repo")

T, B, C = 4, 2, 128
D, H, W = 8, 32, 32
OUT_COLS = D * H * W  # 8192
FILL_COLS = 1024

_COMPILED = {}


def _ensure_trace_hooks():
    """Make trace=True work under axon: register the NTFF profile hook
    (the image's antenv lacks axon_hooks) and keep artifacts local
    (zero-egress container). No-op when tracing is off or already set up."""
    if "antenv.axon_hooks" in sys.modules:
        return
    try:
        import types

        import concourse.bass_utils as bu
        from trn_agent_boot.trn_boot import _ntff_profile_via_ctypes

        bu.upload_artifacts = lambda tmpdir: tmpdir
        hook = _ntff_profile_via_ctypes("/opt/axon/libaxon_pjrt.so")
        mod = types.ModuleType("antenv.axon_hooks")
        mod._hook = hook
        mod.get_axon_ntff_profile_hook = lambda: mod._hook
        mod.set_axon_ntff_profile_hook = lambda h: setattr(mod, "_hook", h)
        sys.modules["antenv.axon_hooks"] = mod
        import antenv

        antenv.axon_hooks = mod
    except Exception:
        pass


def _build():
    import concourse.bacc as bacc
    import concourse.mybir as mybir

    dt = mybir.dt
    nc = bacc.Bacc("TRN2", target_bir_lowering=False, debug=False,
                   enable_asserts=False, num_devices=8)

    fill = nc.dram_tensor("fill", [C, FILL_COLS], dt.float32,
                          kind="ExternalInput")
    out_d = nc.dram_tensor("out", [C, OUT_COLS], dt.float32,
                           kind="ExternalOutput")

    # Raw bass (no TileContext): one fill load, then independent out-DMAs.
    # Manual semaphores; no end-of-kernel cleanup/barrier — the NEFF load
    # zeroes semaphores (the start barrier counts up from 0 on every run),
    # and this kernel executes once per load. Skipping the tile framework
    # avoids its ~250-semaphore postamble sweep and DGE-reset tail (~7us).
    with (
        nc.sbuf_tensor([C, FILL_COLS], dt.float32) as fsb,
        nc.semaphore() as fill_sem,
        nc.semaphore() as out_sem,
    ):
        # load the fill tile as two parallel halves on the two HWDGE queues
        half = FILL_COLS // 2
        nc.sync.dma_start(fsb[:, 0:half], fill[:, 0:half]).then_inc(
            fill_sem, 16)
        nc.scalar.dma_start(fsb[:, half:FILL_COLS],
                            fill[:, half:FILL_COLS]).then_inc(fill_sem, 16)
        # chunk 2 goes DRAM->DRAM on the Pool queue with no dependency —
        # it streams during the window where the other queues still wait
        # on the fill tile, so its read traffic rides a mostly-idle bus.
        # (More DD chunks measured slower in the HBM-contended mode: the
        # extra reads steal write bandwidth chip-wide.)
        nc.gpsimd.dma_start(out_d[:, 2 * FILL_COLS:3 * FILL_COLS],
                            fill[:]).then_inc(out_sem, 16)
        # remaining 7 chunks from SBUF across the 3 dispatch queues; each
        # queue waits for both fill halves, then streams its share. The
        # 360 GB/s/core DMA write bus is the floor.
        qs = {0: nc.sync, 3: nc.sync, 6: nc.sync,
              1: nc.scalar, 4: nc.scalar, 7: nc.scalar,
              5: nc.gpsimd}
        for q in (nc.sync, nc.scalar, nc.gpsimd):
            q.wait_ge(fill_sem, 32)
        for i, q in qs.items():
            q.dma_start(
                out_d[:, i * FILL_COLS:(i + 1) * FILL_COLS],
                fsb[:]).then_inc(out_sem, 16)
        # gate kernel end on every transfer having landed in DRAM
        nc.sync.wait_ge(out_sem, 16 * 8)

    nc.compile()
    return nc


def _in_maps(inputs):
    p_beta = np.ascontiguousarray(np.asarray(inputs["p_beta"], np.float32))
    fill = np.ascontiguousarray(
        np.broadcast_to(p_beta[:, None], (C, FILL_COLS)))
    return [{"fill": fill} for _ in range(8)]


def _assemble(res):
    full = np.empty((T, B, C, D, H, W), np.float32)
    for core in range(8):
        t, b = core // 2, core % 2
        full[t, b] = res.results[core]["out"].reshape(C, D, H, W)
    return full


def kernel(**inputs):
    if os.environ.get("BASS_TRACE"):
        _ensure_trace_hooks()
    from concourse.bass_utils import run_bass_kernel_spmd

    if "nc" not in _COMPILED:
        _COMPILED["nc"] = _build()
    nc = _COMPILED["nc"]

    res = run_bass_kernel_spmd(nc, _in_maps(inputs), core_ids=list(range(8)))
    kernel.last_results = res
    return _assemble(res)
